# revision 6
# baseline (speedup 1.0000x reference)
"""Chamfer loss kernel for Trainium2 (8 NeuronCores, SPMD).

Problem: loss = cd(coarse, gt) + alpha * cd(fine, gt) where
  cd(x, gt) = mean(sqrt(min_x |gt - x|^2)) + 0.1 * mean(sqrt(min_gt |x - gt|^2))

Sharding: core i -> (batch b = i//2, half h = i%2). Each core processes its
half of the query rows (fine: 4096, coarse: 512) against the FULL gt set
(8192) of its batch, block-wise (never materializing the full NxM matrix).

Per core the distance matrix D[q, g] = |q|^2 + |g|^2 - 2 q.g is produced by
the PE via a K=5 augmented matmul:
    W (stationary) = [-2x; -2y; -2z; |q|^2; 1]   (5 x Nq)
    S (moving)     = [  x;   y;   z;    1; |g|^2] (5 x 8192)
Row-mins (per query, over gt) come from a fused DVE tensor_tensor_reduce that
also writes a copy of D into SBUF; col-mins (per gt, over queries) are a
running elementwise min over that copy on GPSIMD, finalized by PE-transpose +
DVE reduce. Host combines the two halves per batch, clamps, sqrts and means.
"""

import os
import sys

import numpy as np

for _p in ("/opt/trn_rl_repo",):
    if _p not in sys.path:
        sys.path.insert(0, _p)

import concourse.bacc as bacc
import concourse.bass as bass
import concourse.tile as tile
from concourse import masks, mybir
from concourse.bass_utils import run_bass_kernel_spmd

F32 = mybir.dt.float32
BIG = 1.0e30


def _install_ntff_hook():
    """The agent image's antenv lacks axon_hooks, which disables NTFF
    profiling under axon. Recreate the module and wire the ctypes hook
    from the boot package so trace=True yields exec_time_ns."""
    try:
        from antenv.axon_hooks import get_axon_ntff_profile_hook  # noqa: F401
        return
    except ImportError:
        pass
    import types

    import antenv

    mod = types.ModuleType("antenv.axon_hooks")
    _holder = {}
    mod.set_axon_ntff_profile_hook = lambda h: _holder.__setitem__("h", h)
    mod.get_axon_ntff_profile_hook = lambda: _holder.get("h")
    sys.modules["antenv.axon_hooks"] = mod
    antenv.axon_hooks = mod
    try:
        if "/root/.axon_site" not in sys.path:
            sys.path.insert(0, "/root/.axon_site")
        from trn_agent_boot.trn_boot import _ntff_profile_via_ctypes
        hook = _ntff_profile_via_ctypes("/opt/axon/libaxon_pjrt.so")
        if hook is not None:
            mod.set_axon_ntff_profile_hook(hook)
    except Exception as e:  # profiling is best-effort; run still works
        print(f"ntff hook install failed: {e}", file=sys.stderr)


_install_ntff_hook()

# Problem constants (hardcoded per contract)
B = 4
NC_PTS = 1024  # coarse points per batch
NF_PTS = 8192  # fine points per batch
NG_PTS = 8192  # gt points per batch
NCORES = 8

# Per-core shard sizes
NF_H = NF_PTS // 2  # 4096
NC_H = NC_PTS // 2  # 512

# Tiling
GRP = 2048          # free-dim columns per DVE/GPSIMD op (4 PSUM banks)
NGRP = NG_PTS // GRP  # 4 groups over gt
FCH = NF_H // 128   # 32 fine chunks
CCH = NC_H // 128   # 4 coarse chunks
TBLK = NG_PTS // 128  # 64 transpose blocks for col-min extraction

# Fraction of groups handled entirely on DVE (tensor_tensor + reduce) instead
# of the DVE-TTR + GPSIMD split; balances DVE vs GPSIMD load.
TYPE_A_MOD = int(os.environ.get("CHAMFER_TYPE_A_MOD", "1"))

# out columns: [0:32] fine row-mins, [32:36] coarse row-mins,
# [36:100] gt-vs-fine col-mins, [100:164] gt-vs-coarse col-mins
OUT_COLS = FCH + CCH + TBLK + TBLK

LAST_EXEC_NS = None
LAST_RESULTS = None

_CACHE = {}


def _build_point_set(nc, tc, pre, psum, dst, dram, npts, identity, ones128,
                     is_query):
    """Fill dst [5, npts] from dram [npts, 3].

    is_query=True  -> rows [-2x, -2y, -2z, |p|^2, 1]
    is_query=False -> rows [x, y, z, 1, |p|^2]
    """
    nchunks = npts // 128
    norm_row, ones_row = (3, 4) if is_query else (4, 3)

    # coordinate rows (strided DMA from DRAM [npts, 3])
    for d in range(3):
        nc.sync.dma_start(out=dst[d:d + 1, :], in_=dram[:, d:d + 1])
    if is_query:
        nc.vector.tensor_scalar_mul(dst[0:3, :], dst[0:3, :], -2.0)

    # ones row
    nc.sync.dma_start(out=dst[ones_row:ones_row + 1, :],
                      in_=ones128[:, 0:nchunks])

    # norms: load [128, nchunks, 3] (partition-major points), square, add,
    # PE-transpose to free-major, DMA into the norm row.
    g128 = pre.tile([128, nchunks, 3], F32, tag="g128")
    nc.sync.dma_start(out=g128, in_=dram.rearrange("(c p) d -> p c d", p=128))
    sq = pre.tile([128, nchunks, 3], F32, tag="sq")
    nc.vector.tensor_mul(sq, g128, g128)
    n128 = pre.tile([128, nchunks], F32, tag="n128")
    nc.vector.tensor_add(n128, sq[:, :, 0], sq[:, :, 1])
    nc.vector.tensor_add(n128, n128, sq[:, :, 2])
    pt = psum.tile([128, 512], F32, tag="grp")
    nc.tensor.transpose(pt[0:nchunks, 0:128], n128, identity)
    tmp = pre.tile([128, 128], F32, tag="tmp")
    nc.vector.tensor_copy(tmp[0:nchunks, :], pt[0:nchunks, 0:128])
    nc.sync.dma_start(out=dst[norm_row:norm_row + 1, :],
                      in_=tmp[0:nchunks, :])


def _build_program():
    if "nc" in _CACHE:
        return _CACHE["nc"]

    nc = bacc.Bacc(None)
    gt_d = nc.declare_dram_parameter("gt", [NG_PTS, 3], F32, isOutput=False)
    fine_d = nc.declare_dram_parameter("fine", [NF_H, 3], F32, isOutput=False)
    coarse_d = nc.declare_dram_parameter("coarse", [NC_H, 3], F32,
                                         isOutput=False)
    out_d = nc.declare_dram_parameter("out", [128, OUT_COLS], F32,
                                      isOutput=True)

    with tile.TileContext(nc) as tc:
        import contextlib
        with contextlib.ExitStack() as ctx:
            singles = ctx.enter_context(tc.tile_pool(name="singles", bufs=1))
            pre = ctx.enter_context(tc.tile_pool(name="pre", bufs=2))
            scr = ctx.enter_context(tc.tile_pool(name="scr", bufs=3))
            rpp = ctx.enter_context(tc.tile_pool(name="rpp", bufs=3))
            psum = ctx.enter_context(
                tc.tile_pool(name="psum", bufs=2, space="PSUM"))

            identity = singles.tile([128, 128], F32)
            masks.make_identity(nc, identity[:])
            ones128 = singles.tile([128, 64], F32)
            nc.gpsimd.memset(ones128[:], 1.0)

            s_gt = singles.tile([5, NG_PTS], F32)
            w_fine = singles.tile([5, NF_H], F32)
            w_coarse = singles.tile([5, NC_H], F32)
            inf_t = singles.tile([128, GRP], F32)
            nc.vector.memset(inf_t[:], BIG)
            m_fine = singles.tile([128, NG_PTS], F32)
            nc.vector.memset(m_fine[:], BIG)
            m_coarse = singles.tile([128, NG_PTS], F32)
            nc.gpsimd.memset(m_coarse[:], BIG)
            rm_fine = singles.tile([128, FCH], F32)
            rm_coarse = singles.tile([128, CCH], F32)
            gt_vs_fine = singles.tile([128, TBLK], F32)
            gt_vs_coarse = singles.tile([128, TBLK], F32)

            _build_point_set(nc, tc, pre, psum, s_gt, gt_d, NG_PTS, identity,
                             ones128, is_query=False)
            _build_point_set(nc, tc, pre, psum, w_fine, fine_d, NF_H, identity,
                             ones128, is_query=True)
            _build_point_set(nc, tc, pre, psum, w_coarse, coarse_d, NC_H,
                             identity, ones128, is_query=True)

            gctr = 0
            for w, nch, m_state, rm in (
                (w_coarse, CCH, m_coarse, rm_coarse),
                (w_fine, FCH, m_fine, rm_fine),
            ):
                for c in range(nch):
                    rp = rpp.tile([128, NGRP], F32, tag="rp")
                    lhsT = w[:, c * 128:(c + 1) * 128]
                    for g in range(NGRP):
                        ps = psum.tile([128, GRP], F32, tag="grp")
                        for j in range(GRP // 512):
                            col = g * GRP + j * 512
                            nc.tensor.matmul(
                                ps[:, j * 512:(j + 1) * 512],
                                lhsT,
                                s_gt[:, col:col + 512],
                                start=True, stop=True,
                            )
                        msl = m_state[:, g * GRP:(g + 1) * GRP]
                        if TYPE_A_MOD > 0 and gctr % TYPE_A_MOD == 0:
                            # all-DVE group
                            nc.vector.tensor_tensor(
                                out=msl, in0=ps[:], in1=msl,
                                op=mybir.AluOpType.min)
                            nc.vector.tensor_reduce(
                                out=rp[:, g:g + 1], in_=ps[:],
                                axis=mybir.AxisListType.X,
                                op=mybir.AluOpType.min)
                        else:
                            # fused: copy D to SBUF + exact row-min in one
                            # DVE pass; col-min update on GPSIMD
                            sc = scr.tile([128, GRP], F32, tag="sc")
                            nc.vector.tensor_tensor_reduce(
                                out=sc[:], in0=ps[:], in1=inf_t[:],
                                scale=1.0, scalar=BIG,
                                op0=mybir.AluOpType.min,
                                op1=mybir.AluOpType.min,
                                accum_out=rp[:, g:g + 1])
                            nc.gpsimd.tensor_tensor(
                                out=msl, in0=sc[:], in1=msl,
                                op=mybir.AluOpType.min)
                        gctr += 1
                    nc.vector.tensor_reduce(
                        out=rm[:, c:c + 1], in_=rp[:],
                        axis=mybir.AxisListType.X, op=mybir.AluOpType.min)

            # col-min extraction: transpose M blocks, reduce over original
            # partitions (=query index) to get per-gt-point mins
            for m_state, gt_min in ((m_coarse, gt_vs_coarse),
                                    (m_fine, gt_vs_fine)):
                for t4 in range(TBLK // 4):
                    pt = psum.tile([128, 512], F32, tag="grp")
                    for j in range(4):
                        t = t4 * 4 + j
                        nc.tensor.transpose(
                            pt[:, j * 128:(j + 1) * 128],
                            m_state[:, t * 128:(t + 1) * 128],
                            identity)
                    nc.vector.tensor_reduce(
                        out=gt_min[:, t4 * 4:(t4 + 1) * 4],
                        in_=pt.rearrange("p (b f) -> p b f", f=128),
                        axis=mybir.AxisListType.X, op=mybir.AluOpType.min)

            c0 = 0
            for t in (rm_fine, rm_coarse, gt_vs_fine, gt_vs_coarse):
                w = t.shape[-1]
                nc.sync.dma_start(out=out_d[:, c0:c0 + w], in_=t[:])
                c0 += w

    nc.finalize()
    _CACHE["nc"] = nc
    return nc


def kernel(coarse, fine, gt, alpha):
    global LAST_EXEC_NS, LAST_RESULTS
    coarse = np.asarray(coarse, dtype=np.float32)
    fine = np.asarray(fine, dtype=np.float32)
    gt = np.asarray(gt, dtype=np.float32)

    nc = _build_program()

    in_maps = []
    for core in range(NCORES):
        b, h = divmod(core, 2)
        in_maps.append({
            "gt": np.ascontiguousarray(gt[b]),
            "fine": np.ascontiguousarray(fine[b, h * NF_H:(h + 1) * NF_H]),
            "coarse": np.ascontiguousarray(coarse[b, h * NC_H:(h + 1) * NC_H]),
        })

    trace = os.environ.get("CHAMFER_TRACE", "0") == "1"
    res = run_bass_kernel_spmd(nc, in_maps, list(range(NCORES)), trace=trace)
    LAST_EXEC_NS = res.exec_time_ns
    LAST_RESULTS = res

    mins_c = np.empty((B, NC_PTS), np.float32)
    mins_f = np.empty((B, NF_PTS), np.float32)
    gmin_f = np.empty((B, NG_PTS), np.float32)
    gmin_c = np.empty((B, NG_PTS), np.float32)
    for core in range(NCORES):
        b, h = divmod(core, 2)
        o = res.results[core]["out"]
        i0 = 0
        rmf = o[:, i0:i0 + FCH].T.reshape(-1); i0 += FCH
        rmc = o[:, i0:i0 + CCH].T.reshape(-1); i0 += CCH
        gf = o[:, i0:i0 + TBLK].T.reshape(-1); i0 += TBLK
        gc = o[:, i0:i0 + TBLK].T.reshape(-1)
        mins_f[b, h * NF_H:(h + 1) * NF_H] = rmf
        mins_c[b, h * NC_H:(h + 1) * NC_H] = rmc
        if h == 0:
            gmin_f[b] = gf
            gmin_c[b] = gc
        else:
            gmin_f[b] = np.minimum(gmin_f[b], gf)
            gmin_c[b] = np.minimum(gmin_c[b], gc)

    def srt(x):
        return np.sqrt(np.maximum(x, 0.0))

    loss_c = srt(gmin_c).mean(dtype=np.float64) \
        + 0.1 * srt(mins_c).mean(dtype=np.float64)
    loss_f = srt(gmin_f).mean(dtype=np.float64) \
        + 0.1 * srt(mins_f).mean(dtype=np.float64)
    return np.float32(loss_c + float(np.asarray(alpha)) * loss_f)
